# revision 54
# baseline (speedup 1.0000x reference)
"""Trainium2 Bass kernel for nn_Decoder (attention-LSTM decoder recurrence).

Math (per batch b, T=128 steps, M=P=64):
    repeat t = 0..T-2:
        e = tanh(H @ U_d.T + W_d @ [h; c])          (T, M)
        s = exp(v_d . e)                            (T,)   softmax numerator
        num = sum_t s_t * (H w~[1:] + w~b + w~0 dec_t)_t
        den = sum_t s_t
        y~  = num / den                             (dec folded into num)
        LSTM(y~, h, c) -> h, c                      (i,f,g,o gates)
    final: attend once more; out = [h, ctx]

Sharding: data-parallel over batch. B=32 over 8 cores -> 4 batches/core.

The recurrence is latency-bound (fixed per-instruction engine latencies),
so the step loop is structured to minimize serial engine stages:
  - e_pre = UH + W q is accumulated in PSUM by matmuls (UH re-materialized
    each step from f16 H, off the critical path; q contribution uses
    stride-0 broadcast rhs), so tanh(e) is ONE bias-free ACT instr.
  - hw_all = H w~[1:] + w~b + w~0 dec[b,t] is host-precomputed, folding
    the y~ add into the num matmul (y~ = num/den, one DVE divide).
  - den matmul issues before the num matmuls so the reciprocal hides
    behind them.
  - gates are one complete 4-matmul PSUM group with lhsT = [Whh; wih; b]
    against [2h; y~; 1] (open accumulation groups on column-separated
    regions of one bank corrupt each other - do not split).
  - gate tanh outputs land next to the 2c state in one f16 tile so
    (ti+1)*tg and (tf+1)*2c fuse into a single 8-column DVE op.
State stores 2h / 2c (sigmoid(z) = 0.5 tanh(0.5 z) + 0.5 folding; the 0.5s
live in the host-packed weights).
"""

import numpy as np

B, T, M, P = 32, 128, 64, 64
NCORES = 8
BL = B // NCORES          # batches per core = 4
NG = 2                    # attention groups per core
GB = BL // NG             # batches per attention group = 2

_STATE = {}
NSTEPS = T - 1
DEBUG = False
PSTATE_PAD = False    # PE p-state padding: measured no clock ramp on HW


def _build_nc():
    import contextlib

    import concourse.bacc as bacc
    import concourse.tile as tile
    from concourse import mybir

    f32 = mybir.dt.float32
    f32r = mybir.dt.float32r
    f16 = mybir.dt.float16
    AF = mybir.ActivationFunctionType
    OP = mybir.AluOpType

    nc = bacc.Bacc()

    # ---- per-core sharded data ----
    h_l = nc.declare_dram_parameter("h_l", [BL, T, M], f32, isOutput=False)
    hw_all = nc.declare_dram_parameter("hw_all", [T, 4 * T], f16, isOutput=False)
    # pk packs state + all weights + H^T into ONE DMA (each separate DMA
    # costs ~1.9us of SWDGE setup + completion-sem latency in the preamble):
    # cols 0:8 [2h|2c] (rows 0:64), 8:10 v2, 10:138 wd2 (rows 0:64),
    # 138:202 udT (rows 0:64), 202:458 whbi (rows 0:66),
    # 458:714 htp0, 714:970 htp1 (rows 0:64)
    PKC = 970
    pk = nc.declare_dram_parameter("pk", [2 * P, PKC], f16, isOutput=False)
    # ---- output: [ctx | den] in cols 0:4, 2h in cols 4:8 (one DMA) ----
    oall = nc.declare_dram_parameter("oall", [P + 1, 2 * BL], f32,
                                     isOutput=True)
    if DEBUG:
        dbg_g = nc.declare_dram_parameter("dbg_g", [P, 4 * BL], f32, isOutput=True)

    with tile.TileContext(nc) as tc:
        with contextlib.ExitStack() as ctx:
            consts = ctx.enter_context(tc.tile_pool(name="consts", bufs=1))
            state = ctx.enter_context(tc.tile_pool(name="state", bufs=1))
            loop_sb = ctx.enter_context(tc.tile_pool(name="loop_sb", bufs=3))
            loop_ps = ctx.enter_context(
                tc.tile_pool(name="loop_ps", bufs=1, space="PSUM")
            )
            ep_pool = ctx.enter_context(
                tc.tile_pool(name="ep_ps", bufs=1, space="PSUM")
            )

            # -------- preamble: ONE packed DMA for state+weights+H^T;
            # hw_all/haug (final-ctx only) ride the gpsimd queue --------
            pk_sb = consts.tile([2 * P, PKC], f16)
            nc.sync.dma_start(out=pk_sb, in_=pk[:])
            v2_sb = pk_sb[:, 8:10]
            wd2_sb = pk_sb[0:P, 10:138]
            udT_sb = pk_sb[0:P, 138:202]
            whbi_sb = pk_sb[0:P + 2, 202:458]
            htp_sb = [pk_sb[0:P, 458:714], pk_sb[0:P, 714:970]]
            hw_sb = consts.tile([T, 4 * T], f16)
            nc.gpsimd.dma_start(out=hw_sb, in_=hw_all[:])
            haug = []
            for b in range(BL):
                hb = consts.tile([T, M], f32r, tag=f"HAUG{b}")
                nc.gpsimd.dma_start(out=hb, in_=h_l[b].bitcast(f32r))
                haug.append(hb)
            ones_f = consts.tile([T, 1], f32)
            nc.vector.memset(ones_f, 1.0)
            ones16 = consts.tile([T, 1], f16)
            nc.vector.tensor_copy(out=ones16, in_=ones_f)

            # sy2 = [2h (0:64); y~ (64); 1 (65)]; tgc packs the gate tanh
            # outputs [i,f,o,g] (cols 0:16) next to the 2c state (cols
            # 16:20) so (ti+1)*tg and (tf+1)*2c fuse into ONE 8-column
            # DVE op: in0=[i,f], in1=[g,2c]
            sy2 = state.tile([P + 2, BL], f16, tag="SY")
            nc.vector.tensor_copy(out=sy2[0:P, :], in_=pk_sb[0:P, 0:4])
            tgc = state.tile([P, 5 * BL], f16, tag="TGC")
            cs = tgc[:, 4 * BL:5 * BL]
            nc.vector.tensor_copy(out=cs, in_=pk_sb[0:P, 4:8])
            ones_g = state.tile([P + 2, BL], f32, tag="ONESG")
            nc.vector.memset(ones_g[P:P + 2, :], 1.0)
            nc.vector.tensor_copy(out=sy2[P:P + 2, :], in_=ones_g[P:P + 2, :])

            # scratch bank for p-state padding matmuls (write-only)
            if PSTATE_PAD:
                pad_ps = ctx.enter_context(
                    tc.tile_pool(name="pad_ps", bufs=1, space="PSUM"))

            def pad(n):
                # dummy matmuls with no waits: they fill PE idle windows so
                # the tensor engine stays busy and ramps to full clock
                if not PSTATE_PAD:
                    return
                for _ in range(n):
                    dtile = pad_ps.tile([M, NG * T], f32, tag="PAD")
                    nc.tensor.matmul(dtile, udT_sb, htp_sb[0],
                                     start=True, stop=True)

            # Two independent 2-batch chains (g=0: batches 0,1; g=1: 2,3).
            # In-order engines pipeline the chains ~half a step apart, so
            # the wall period approaches per-engine work instead of the
            # serial single-chain latency.
            tgr = tgc.rearrange("p (k b) -> p k b", b=BL)

            def attn(t, g):
                cs_g = tgc[:, 4 * BL + 2 * g:4 * BL + 2 * g + 2]
                ep = ep_pool.tile([2 * M, T], f32, tag=f"EP{g}",
                                  name=f"ep{g}")
                for j in range(GB):
                    nc.tensor.matmul(
                        ep[j * M:(j + 1) * M, :], udT_sb,
                        htp_sb[j][:, g * T:(g + 1) * T],
                        start=True, stop=False)
                for j in range(GB):
                    nc.tensor.matmul(
                        ep[j * M:(j + 1) * M, :], wd2_sb[:, 0:M],
                        cs_g[:, j:j + 1].to_broadcast([P, T]),
                        start=False, stop=False, skip_group_check=True)
                for j in range(GB):
                    nc.tensor.matmul(
                        ep[j * M:(j + 1) * M, :], wd2_sb[:, M:2 * M],
                        sy2[0:P, 2 * g + j:2 * g + j + 1].to_broadcast(
                            [P, T]),
                        start=False, stop=True, skip_group_check=True)
                e_sb = loop_sb.tile([2 * M, T], f16, tag=f"e{g}",
                                    name=f"e{g}")
                nc.scalar.activation(out=e_sb, in_=ep, func=AF.Tanh)
                # lg (cols 0:2) + den (row 64, cols 2:4) + num (cols 4:6)
                # share one PSUM bank tile per chain
                ldn = loop_ps.tile([T, 3 * GB], f32, tag=f"LDN{g}",
                                   name=f"ldn{g}")
                nc.tensor.matmul(ldn[:, 0:GB], e_sb, v2_sb,
                                 start=True, stop=True)
                s_sb = loop_sb.tile([T, GB], f16, tag=f"s{g}", name=f"s{g}")
                nc.scalar.activation(out=s_sb, in_=ldn[:, 0:GB], func=AF.Exp)
                nc.tensor.matmul(ldn[P:P + 1, GB:2 * GB], ones16, s_sb,
                                 start=True, stop=True)
                if t is not None:
                    for j in range(GB):
                        c0 = 4 * t + 2 * g + j
                        nc.tensor.matmul(
                            ldn[P:P + 1, 2 * GB + j:2 * GB + j + 1],
                            hw_sb[:, c0:c0 + 1], s_sb[:, j:j + 1],
                            start=True, stop=True)
                return ldn, s_sb

            def lstm(g, ldn):
                bs = slice(2 * g, 2 * g + 2)
                rden = loop_sb.tile([P + 2, GB], f32, tag=f"rd{g}",
                                    name=f"rd{g}")
                nc.vector.reciprocal(
                    out=rden[P:P + 1, :], in_=ldn[P:P + 1, GB:2 * GB])
                nc.vector.tensor_tensor(
                    out=sy2[P:P + 1, bs], in0=ldn[P:P + 1, 2 * GB:3 * GB],
                    in1=rden[P:P + 1, :], op=OP.mult)
                gps = loop_ps.tile([P, 4 * GB], f32, tag=f"G{g}",
                                   name=f"g{g}")
                for k in range(4):
                    nc.tensor.matmul(
                        gps[:, k * GB:(k + 1) * GB],
                        whbi_sb[:, k * P:(k + 1) * P], sy2[:, bs],
                        start=True, stop=True)
                nc.scalar.activation(
                    out=tgr[:, 0:4, bs],
                    in_=gps.rearrange("p (k b) -> p k b", b=GB),
                    func=AF.Tanh)
                sUV = loop_sb.tile([P, 2 * GB], f16, tag=f"sUV{g}",
                                   name=f"suv{g}")
                suvr = sUV.rearrange("p (k b) -> p k b", b=GB)
                nc.vector.scalar_tensor_tensor(
                    out=suvr, in0=tgr[:, 0:2, bs], scalar=1.0,
                    in1=tgr[:, 3:5, bs], op0=OP.add, op1=OP.mult)
                nc.vector.scalar_tensor_tensor(
                    out=tgr[:, 4:5, bs], in0=suvr[:, 1:2, :],
                    in1=suvr[:, 0:1, :], scalar=0.5,
                    op0=OP.mult, op1=OP.add)
                th = loop_sb.tile([P, GB], f32, tag=f"th{g}",
                                  name=f"th{g}")
                nc.scalar.activation(
                    out=th, in_=tgc[:, 4 * BL + 2 * g:4 * BL + 2 * g + 2],
                    func=AF.Tanh, scale=0.5)
                nc.vector.scalar_tensor_tensor(
                    out=sy2[0:P, bs], in0=tgr[:, 2, bs], scalar=1.0,
                    in1=th, op0=OP.add, op1=OP.mult)

            # ---------------- main recurrence ----------------
            for t in range(NSTEPS):
                for g in range(NG):
                    ldn, _ = attn(t, g)
                    lstm(g, ldn)

            # ---------------- final attend + outputs ----------------
            ctx_ps = loop_ps.tile([M, 2 * BL], f32, tag="CTXF")
            ob = loop_sb.tile([P + 2, 2 * BL], f32, tag="ob")
            for g in range(NG):
                ldn, s_fin = attn(None, g)
                s_fr = loop_sb.tile([T, GB], f32r, tag=f"sfr{g}",
                                    name=f"sfr{g}")
                nc.vector.tensor_copy(out=s_fr, in_=s_fin)
                for j in range(GB):
                    b = 2 * g + j
                    nc.tensor.matmul(
                        ctx_ps[:, 2 * b:2 * b + 2], haug[b],
                        s_fr[:, j:j + 1].to_broadcast([T, 2]),
                        start=True, stop=True)
                nc.vector.tensor_copy(
                    out=ob[P:P + 1, 2 * g:2 * g + 2],
                    in_=ldn[P:P + 1, GB:2 * GB])
            nc.vector.tensor_copy(
                out=ob[0:M, 0:BL],
                in_=ctx_ps.rearrange("p (b two) -> p b two", two=2)[:, :, 0])
            nc.vector.tensor_copy(out=ob[0:P, BL:2 * BL], in_=sy2[0:P, :])
            nc.sync.dma_start(out=oall[:], in_=ob[0:P + 1, :])

    nc.finalize()
    return nc


def _pack_weights(W_d, U_d, v_d, w_tilde_W, w_tilde_b, W_ih, W_hh, b_ih, b_hh):
    f16 = np.float16
    # q = [h;c] stored as 2h;2c -> fold 0.5 into W_d^T halves
    # wd2 cols 0:M = c-half lhsT, cols M:2M = h-half lhsT
    wd2 = np.zeros((P, 2 * M), dtype=f16)
    wd2[:, 0:M] = 0.5 * W_d[:, P:2 * P].T
    wd2[:, M:2 * M] = 0.5 * W_d[:, 0:P].T
    udT16 = np.ascontiguousarray(U_d.T, dtype=f16)
    v2 = np.zeros((2 * M, GB), dtype=f16)
    v2[0:M, 0] = v_d[0]
    v2[M:2 * M, 1] = v_d[0]
    bsum = (b_ih + b_hh).astype(np.float32)
    wih = W_ih[:, 0].astype(np.float32)
    # torch gate order i,f,g,o; our column order i,f,o,g.
    # sigmoid gates (i,f,o): pre-scale 0.5 (sigmoid(z) = 0.5 tanh(0.5 z)+0.5)
    # h input is 2h -> extra 0.5 on W_hh blocks.
    src = [0, 1, 3, 2]                    # i, f, o, g row-blocks in torch order
    sig = [0.5, 0.5, 0.5, 1.0]
    whbi = np.zeros((P + 2, 4 * P), dtype=f16)
    for k in range(4):
        blk = slice(src[k] * P, (src[k] + 1) * P)
        whbi[0:P, k * P:(k + 1) * P] = sig[k] * 0.5 * W_hh[blk].T
        whbi[P, k * P:(k + 1) * P] = sig[k] * wih[blk]
        whbi[P + 1, k * P:(k + 1) * P] = sig[k] * bsum[blk]
    return dict(wd2=wd2, udT16=udT16, v2=v2, whbi=whbi)


def kernel(H, dec_data, d_1, s_1, W_d, U_d, v_d, w_tilde_W, w_tilde_b,
           W_ih, W_hh, b_ih, b_hh, T=None):
    from concourse.bass_utils import run_bass_kernel_spmd

    H = np.asarray(H, dtype=np.float32)
    dec_data = np.asarray(dec_data, dtype=np.float32)
    d_1 = np.asarray(d_1, dtype=np.float32)
    s_1 = np.asarray(s_1, dtype=np.float32)
    W_d = np.asarray(W_d, np.float32)
    w_tilde_W = np.asarray(w_tilde_W, np.float32)
    w_tilde_b = np.asarray(w_tilde_b, np.float32)

    if "nc" not in _STATE:
        _STATE["nc"] = _build_nc()
    nc = _STATE["nc"]

    wpack = _pack_weights(
        W_d, np.asarray(U_d, np.float32),
        np.asarray(v_d, np.float32), w_tilde_W, w_tilde_b,
        np.asarray(W_ih, np.float32), np.asarray(W_hh, np.float32),
        np.asarray(b_ih, np.float32), np.asarray(b_hh, np.float32),
    )

    wt1 = w_tilde_W[0, 1:M + 1]                         # [64]
    w0 = w_tilde_W[0, 0]
    wtb = w_tilde_b[0]

    # shared weight block of the packed-constant DMA (cols 8:458)
    pk_t = np.zeros((2 * P, 970), dtype=np.float16)
    pk_t[:, 8:10] = wpack["v2"]
    pk_t[0:P, 10:138] = wpack["wd2"]
    pk_t[0:P, 138:202] = wpack["udT16"]
    pk_t[0:P + 2, 202:458] = wpack["whbi"]

    in_maps = []
    for core in range(NCORES):
        sl = slice(core * BL, (core + 1) * BL)
        h_c = H[sl]                                     # [4, T, M]
        pk = pk_t.copy()
        # state: cols 0:4 = 2h, 4:8 = 2c
        pk[0:P, 0:4] = 2.0 * d_1[0, sl].T
        pk[0:P, 4:8] = 2.0 * s_1[0, sl].T
        # htp[j][m, g*T + t] = H[2g+j][t, m] at cols 458 + j*256
        for j in range(GB):
            for g in range(NG):
                c0 = 458 + j * 256 + g * 128
                pk[0:P, c0:c0 + 128] = h_c[NG * g + j].T
        # hw_all[t, 4*s + b] = (H_b @ wt1)[t] + wtb + w0*dec[b, s]
        hwb = h_c @ wt1 + wtb                           # [4, T]
        dec_c = dec_data[sl, :, 0]                      # [4, T]
        # X[s, b, t] = hwb[b, t] + w0*dec[b, s]; want hw_all[t, 4*s+b]
        hw_all = (hwb[None, :, :] + (w0 * dec_c).T[:, :, None]
                  ).transpose(2, 0, 1).reshape(128, 4 * 128)
        m = dict(
            h_l=np.ascontiguousarray(h_c),
            hw_all=np.ascontiguousarray(hw_all.astype(np.float16)),
            pk=np.ascontiguousarray(pk),
        )
        in_maps.append(m)

    res = run_bass_kernel_spmd(nc, in_maps, list(range(NCORES)))
    _STATE["last_results"] = res

    out = np.zeros((B, 1, P + M), dtype=np.float32)
    for core in range(NCORES):
        r = res.results[core]["oall"]             # [65, 8]
        hv = r[0:P, BL:2 * BL].T * 0.5            # [4, 64]  (state was 2h)
        ctxv = (r[0:M, 0:BL] / r[M:M + 1, 0:BL]).T  # [4, 64]
        out[core * BL:(core + 1) * BL, 0, 0:P] = hv
        out[core * BL:(core + 1) * BL, 0, P:P + M] = ctxv
    return out


# revision 55
# speedup vs baseline: 1.0277x; 1.0277x over previous
"""Trainium2 Bass kernel for nn_Decoder (attention-LSTM decoder recurrence).

Math (per batch b, T=128 steps, M=P=64):
    repeat t = 0..T-2:
        e = tanh(H @ U_d.T + W_d @ [h; c])          (T, M)
        s = exp(v_d . e)                            (T,)   softmax numerator
        num = sum_t s_t * (H w~[1:] + w~b + w~0 dec_t)_t
        den = sum_t s_t
        y~  = num / den                             (dec folded into num)
        LSTM(y~, h, c) -> h, c                      (i,f,g,o gates)
    final: attend once more; out = [h, ctx]

Sharding: data-parallel over batch. B=32 over 8 cores -> 4 batches/core.

The recurrence is latency-bound (fixed per-instruction engine latencies),
so the step loop is structured to minimize serial engine stages:
  - e_pre = UH + W q is accumulated in PSUM by matmuls (UH re-materialized
    each step from f16 H, off the critical path; q contribution uses
    stride-0 broadcast rhs), so tanh(e) is ONE bias-free ACT instr.
  - hw_all = H w~[1:] + w~b + w~0 dec[b,t] is host-precomputed, folding
    the y~ add into the num matmul (y~ = num/den, one DVE divide).
  - den matmul issues before the num matmuls so the reciprocal hides
    behind them.
  - gates are one complete 4-matmul PSUM group with lhsT = [Whh; wih; b]
    against [2h; y~; 1] (open accumulation groups on column-separated
    regions of one bank corrupt each other - do not split).
  - gate tanh outputs land next to the 2c state in one f16 tile so
    (ti+1)*tg and (tf+1)*2c fuse into a single 8-column DVE op.
State stores 2h / 2c (sigmoid(z) = 0.5 tanh(0.5 z) + 0.5 folding; the 0.5s
live in the host-packed weights).
"""

import numpy as np

B, T, M, P = 32, 128, 64, 64
NCORES = 8
BL = B // NCORES          # batches per core = 4
NG = 2                    # attention groups per core
GB = BL // NG             # batches per attention group = 2

_STATE = {}
NSTEPS = T - 1
DEBUG = False
PSTATE_PAD = False    # PE p-state padding: measured no clock ramp on HW


def _build_nc():
    import contextlib

    import concourse.bacc as bacc
    import concourse.tile as tile
    from concourse import mybir

    f32 = mybir.dt.float32
    f32r = mybir.dt.float32r
    f16 = mybir.dt.float16
    AF = mybir.ActivationFunctionType
    OP = mybir.AluOpType

    nc = bacc.Bacc()

    # ---- per-core sharded data ----
    h_l = nc.declare_dram_parameter("h_l", [BL, T, M], f32, isOutput=False)
    hw_all = nc.declare_dram_parameter("hw_all", [T, 4 * T], f16, isOutput=False)
    # pk packs state + all weights + H^T into ONE DMA (each separate DMA
    # costs ~1.9us of SWDGE setup + completion-sem latency in the preamble):
    # cols 0:8 [2h|2c] (rows 0:64), 8:10 v2, 10:138 wd2 (rows 0:64),
    # 138:202 udT (rows 0:64), 202:458 whbi (rows 0:66),
    # 458:714 htp0, 714:970 htp1 (rows 0:64)
    PKC = 970
    pk = nc.declare_dram_parameter("pk", [2 * P, PKC], f16, isOutput=False)
    # ---- output: [ctx | den] in cols 0:4, 2h in cols 4:8 (one DMA) ----
    oall = nc.declare_dram_parameter("oall", [P + 1, 2 * BL], f32,
                                     isOutput=True)
    if DEBUG:
        dbg_g = nc.declare_dram_parameter("dbg_g", [P, 4 * BL], f32, isOutput=True)

    with tile.TileContext(nc) as tc:
        with contextlib.ExitStack() as ctx:
            consts = ctx.enter_context(tc.tile_pool(name="consts", bufs=1))
            state = ctx.enter_context(tc.tile_pool(name="state", bufs=1))
            loop_sb = ctx.enter_context(tc.tile_pool(name="loop_sb", bufs=3))
            loop_ps = ctx.enter_context(
                tc.tile_pool(name="loop_ps", bufs=1, space="PSUM")
            )
            ep_pool = ctx.enter_context(
                tc.tile_pool(name="ep_ps", bufs=2, space="PSUM")
            )

            # -------- preamble: ONE packed DMA for state+weights+H^T;
            # hw_all/haug (final-ctx only) ride the gpsimd queue --------
            pk_sb = consts.tile([2 * P, PKC], f16)
            nc.sync.dma_start(out=pk_sb, in_=pk[:])
            v2_sb = pk_sb[:, 8:10]
            wd2_sb = pk_sb[0:P, 10:138]
            udT_sb = pk_sb[0:P, 138:202]
            whbi_sb = pk_sb[0:P + 2, 202:458]
            htp_sb = [pk_sb[0:P, 458:714], pk_sb[0:P, 714:970]]
            hw_sb = consts.tile([T, 4 * T], f16)
            nc.gpsimd.dma_start(out=hw_sb, in_=hw_all[:])
            haug = []
            for b in range(BL):
                hb = consts.tile([T, M], f32r, tag=f"HAUG{b}")
                nc.gpsimd.dma_start(out=hb, in_=h_l[b].bitcast(f32r))
                haug.append(hb)
            ones_f = consts.tile([T, 1], f32)
            nc.vector.memset(ones_f, 1.0)
            ones16 = consts.tile([T, 1], f16)
            nc.vector.tensor_copy(out=ones16, in_=ones_f)

            # sy2 = [2h (0:64); y~ (64); 1 (65)]; tgc packs the gate tanh
            # outputs [i,f,o,g] (cols 0:16) next to the 2c state (cols
            # 16:20) so (ti+1)*tg and (tf+1)*2c fuse into ONE 8-column
            # DVE op: in0=[i,f], in1=[g,2c]
            sy2 = state.tile([P + 2, BL], f16, tag="SY")
            nc.vector.tensor_copy(out=sy2[0:P, :], in_=pk_sb[0:P, 0:4])
            tgc = state.tile([P, 5 * BL], f16, tag="TGC")
            cs = tgc[:, 4 * BL:5 * BL]
            nc.vector.tensor_copy(out=cs, in_=pk_sb[0:P, 4:8])
            ones_g = state.tile([P + 2, BL], f32, tag="ONESG")
            nc.vector.memset(ones_g[P:P + 2, :], 1.0)
            nc.vector.tensor_copy(out=sy2[P:P + 2, :], in_=ones_g[P:P + 2, :])

            # scratch bank for p-state padding matmuls (write-only)
            if PSTATE_PAD:
                pad_ps = ctx.enter_context(
                    tc.tile_pool(name="pad_ps", bufs=1, space="PSUM"))

            def pad(n):
                # dummy matmuls with no waits: they fill PE idle windows so
                # the tensor engine stays busy and ramps to full clock
                if not PSTATE_PAD:
                    return
                for _ in range(n):
                    dtile = pad_ps.tile([M, NG * T], f32, tag="PAD")
                    nc.tensor.matmul(dtile, udT_sb, htp_sb[0],
                                     start=True, stop=True)

            # ---------- one step's attention front: e_pre..num/den ----------
            def attention(t, need_num=True):
                ep = ep_pool.tile([2 * M, NG * T], f32, tag="EP")
                epr = ep.rearrange("p (g t) -> p g t", g=NG)
                # UH accumulation (consts only -> runs during prev LSTM tail)
                for j in range(GB):
                    nc.tensor.matmul(
                        ep[j * M:(j + 1) * M, :], udT_sb, htp_sb[j],
                        start=True, stop=False)
                # qW c-half then h-half, broadcast over t
                csr = cs.rearrange("p (g j) -> p g j", j=GB)
                for j in range(GB):
                    nc.tensor.matmul(
                        epr[j * M:(j + 1) * M, :, :], wd2_sb[:, 0:M],
                        csr[:, :, j].to_broadcast([P, NG, T]),
                        start=False, stop=False, skip_group_check=True)
                syr = sy2[0:P, :].rearrange("p (g j) -> p g j", j=GB)
                for j in range(GB):
                    nc.tensor.matmul(
                        epr[j * M:(j + 1) * M, :, :], wd2_sb[:, M:2 * M],
                        syr[:, :, j].to_broadcast([P, NG, T]),
                        start=False, stop=True, skip_group_check=True)
                # e = tanh(e_pre): single bias-free ACT instr
                e_sb = loop_sb.tile([2 * M, NG * T], f16, tag="e")
                nc.scalar.activation(out=e_sb, in_=ep, func=AF.Tanh)
                # logits
                lg = loop_ps.tile([T, 2 * GB], f32, tag="LG")
                for g in range(NG):
                    nc.tensor.matmul(
                        lg[:, g * GB:(g + 1) * GB],
                        e_sb[:, g * T:(g + 1) * T], v2_sb,
                        start=True, stop=True)
                # softmax numerators
                s_sb = loop_sb.tile([T, BL], f16, tag="s")
                nc.scalar.activation(out=s_sb, in_=lg, func=AF.Exp)
                # den first (its reciprocal overlaps the num matmuls);
                # scalar row-world lives at partition 64 (aligns with the
                # y~ slot in sy2 for the DVE ops)
                dn = loop_ps.tile([P + 2, 2 * BL], f32, tag="DN")
                nc.tensor.matmul(dn[P:P + 1, 0:BL], ones16, s_sb,
                                 start=True, stop=True)
                if need_num:
                    for b in range(BL):
                        nc.tensor.matmul(
                            dn[P:P + 1, BL + b:BL + b + 1],
                            hw_sb[:, 4 * t + b:4 * t + b + 1],
                            s_sb[:, b:b + 1], start=True, stop=True)
                pad(1)
                return dn, s_sb

            # ---------------- main recurrence ----------------
            for t in range(NSTEPS):
                dn, _ = attention(t)
                # y~ = num * (1/den); recip issues right after the den
                # matmul and overlaps the num matmuls; y~ lands in sy2
                rden = loop_sb.tile([P + 2, BL], f32, tag="rden")
                nc.vector.reciprocal(
                    out=rden[P:P + 1, :], in_=dn[P:P + 1, 0:BL])
                nc.vector.tensor_tensor(
                    out=sy2[P:P + 1, :], in0=dn[P:P + 1, BL:2 * BL],
                    in1=rden[P:P + 1, :], op=OP.mult)
                # gates: single complete group, lhsT = [Whh; wih; b]
                gps = loop_ps.tile([P, 4 * BL], f32, tag="G")
                for k in range(4):
                    nc.tensor.matmul(
                        gps[:, k * BL:(k + 1) * BL],
                        whbi_sb[:, k * P:(k + 1) * P], sy2,
                        start=True, stop=True)
                nc.scalar.activation(out=tgc[:, 0:4 * BL], in_=gps,
                                     func=AF.Tanh)
                if DEBUG and t == 0:
                    g32 = loop_sb.tile([P, 4 * BL], f32, tag="g32")
                    nc.vector.tensor_copy(out=g32, in_=gps)
                    nc.sync.dma_start(out=dbg_g[:], in_=g32)
                # tgc cols: [i, f, o, g, 2c]; 2c_new = 0.5*(tf+1)*2c
                # + (ti+1)*tg: one fused 8-col op gives [sV, sU], then
                # 2c_new = 0.5*sU + sV
                sUV = loop_sb.tile([P, 2 * BL], f16, tag="sUV")
                nc.vector.scalar_tensor_tensor(
                    out=sUV, in0=tgc[:, 0:2 * BL], scalar=1.0,
                    in1=tgc[:, 3 * BL:5 * BL], op0=OP.add, op1=OP.mult)
                nc.vector.scalar_tensor_tensor(
                    out=cs, in0=sUV[:, BL:2 * BL], in1=sUV[:, 0:BL],
                    scalar=0.5, op0=OP.mult, op1=OP.add)
                th = loop_sb.tile([P, BL], f32, tag="th")
                nc.scalar.activation(out=th, in_=cs, func=AF.Tanh, scale=0.5)
                # 2h_new = (to+1)*th
                nc.vector.scalar_tensor_tensor(
                    out=sy2[0:P, :], in0=tgc[:, 2 * BL:3 * BL], scalar=1.0,
                    in1=th, op0=OP.add, op1=OP.mult)

            # ---------------- final attend + outputs ----------------
            dn, s_fin = attention(None, need_num=False)
            s_fr = loop_sb.tile([T, BL], f32r, tag="sfr")
            nc.vector.tensor_copy(out=s_fr, in_=s_fin)
            ctx_ps = loop_ps.tile([M, 2 * BL], f32, tag="CTXF")
            for b in range(BL):
                nc.tensor.matmul(
                    ctx_ps[:, 2 * b:2 * b + 2], haug[b],
                    s_fr[:, b:b + 1].to_broadcast([T, 2]),
                    start=True, stop=True)
            ob = loop_sb.tile([P + 2, 2 * BL], f32, tag="ob")
            nc.vector.tensor_copy(
                out=ob[0:M, 0:BL],
                in_=ctx_ps.rearrange("p (b two) -> p b two", two=2)[:, :, 0])
            nc.vector.tensor_copy(
                out=ob[P:P + 1, 0:BL], in_=dn[P:P + 1, 0:BL])
            nc.vector.tensor_copy(out=ob[0:P, BL:2 * BL], in_=sy2[0:P, :])
            nc.sync.dma_start(out=oall[:], in_=ob[0:P + 1, :])

    nc.finalize()
    return nc


def _pack_weights(W_d, U_d, v_d, w_tilde_W, w_tilde_b, W_ih, W_hh, b_ih, b_hh):
    f16 = np.float16
    # q = [h;c] stored as 2h;2c -> fold 0.5 into W_d^T halves
    # wd2 cols 0:M = c-half lhsT, cols M:2M = h-half lhsT
    wd2 = np.zeros((P, 2 * M), dtype=f16)
    wd2[:, 0:M] = 0.5 * W_d[:, P:2 * P].T
    wd2[:, M:2 * M] = 0.5 * W_d[:, 0:P].T
    udT16 = np.ascontiguousarray(U_d.T, dtype=f16)
    v2 = np.zeros((2 * M, GB), dtype=f16)
    v2[0:M, 0] = v_d[0]
    v2[M:2 * M, 1] = v_d[0]
    bsum = (b_ih + b_hh).astype(np.float32)
    wih = W_ih[:, 0].astype(np.float32)
    # torch gate order i,f,g,o; our column order i,f,o,g.
    # sigmoid gates (i,f,o): pre-scale 0.5 (sigmoid(z) = 0.5 tanh(0.5 z)+0.5)
    # h input is 2h -> extra 0.5 on W_hh blocks.
    src = [0, 1, 3, 2]                    # i, f, o, g row-blocks in torch order
    sig = [0.5, 0.5, 0.5, 1.0]
    whbi = np.zeros((P + 2, 4 * P), dtype=f16)
    for k in range(4):
        blk = slice(src[k] * P, (src[k] + 1) * P)
        whbi[0:P, k * P:(k + 1) * P] = sig[k] * 0.5 * W_hh[blk].T
        whbi[P, k * P:(k + 1) * P] = sig[k] * wih[blk]
        whbi[P + 1, k * P:(k + 1) * P] = sig[k] * bsum[blk]
    return dict(wd2=wd2, udT16=udT16, v2=v2, whbi=whbi)


def kernel(H, dec_data, d_1, s_1, W_d, U_d, v_d, w_tilde_W, w_tilde_b,
           W_ih, W_hh, b_ih, b_hh, T=None):
    from concourse.bass_utils import run_bass_kernel_spmd

    H = np.asarray(H, dtype=np.float32)
    dec_data = np.asarray(dec_data, dtype=np.float32)
    d_1 = np.asarray(d_1, dtype=np.float32)
    s_1 = np.asarray(s_1, dtype=np.float32)
    W_d = np.asarray(W_d, np.float32)
    w_tilde_W = np.asarray(w_tilde_W, np.float32)
    w_tilde_b = np.asarray(w_tilde_b, np.float32)

    if "nc" not in _STATE:
        _STATE["nc"] = _build_nc()
    nc = _STATE["nc"]

    wpack = _pack_weights(
        W_d, np.asarray(U_d, np.float32),
        np.asarray(v_d, np.float32), w_tilde_W, w_tilde_b,
        np.asarray(W_ih, np.float32), np.asarray(W_hh, np.float32),
        np.asarray(b_ih, np.float32), np.asarray(b_hh, np.float32),
    )

    wt1 = w_tilde_W[0, 1:M + 1]                         # [64]
    w0 = w_tilde_W[0, 0]
    wtb = w_tilde_b[0]

    # shared weight block of the packed-constant DMA (cols 8:458)
    pk_t = np.zeros((2 * P, 970), dtype=np.float16)
    pk_t[:, 8:10] = wpack["v2"]
    pk_t[0:P, 10:138] = wpack["wd2"]
    pk_t[0:P, 138:202] = wpack["udT16"]
    pk_t[0:P + 2, 202:458] = wpack["whbi"]

    in_maps = []
    for core in range(NCORES):
        sl = slice(core * BL, (core + 1) * BL)
        h_c = H[sl]                                     # [4, T, M]
        pk = pk_t.copy()
        # state: cols 0:4 = 2h, 4:8 = 2c
        pk[0:P, 0:4] = 2.0 * d_1[0, sl].T
        pk[0:P, 4:8] = 2.0 * s_1[0, sl].T
        # htp[j][m, g*T + t] = H[2g+j][t, m] at cols 458 + j*256
        for j in range(GB):
            for g in range(NG):
                c0 = 458 + j * 256 + g * 128
                pk[0:P, c0:c0 + 128] = h_c[NG * g + j].T
        # hw_all[t, 4*s + b] = (H_b @ wt1)[t] + wtb + w0*dec[b, s]
        hwb = h_c @ wt1 + wtb                           # [4, T]
        dec_c = dec_data[sl, :, 0]                      # [4, T]
        # X[s, b, t] = hwb[b, t] + w0*dec[b, s]; want hw_all[t, 4*s+b]
        hw_all = (hwb[None, :, :] + (w0 * dec_c).T[:, :, None]
                  ).transpose(2, 0, 1).reshape(128, 4 * 128)
        m = dict(
            h_l=np.ascontiguousarray(h_c),
            hw_all=np.ascontiguousarray(hw_all.astype(np.float16)),
            pk=np.ascontiguousarray(pk),
        )
        in_maps.append(m)

    res = run_bass_kernel_spmd(nc, in_maps, list(range(NCORES)))
    _STATE["last_results"] = res

    out = np.zeros((B, 1, P + M), dtype=np.float32)
    for core in range(NCORES):
        r = res.results[core]["oall"]             # [65, 8]
        hv = r[0:P, BL:2 * BL].T * 0.5            # [4, 64]  (state was 2h)
        ctxv = (r[0:M, 0:BL] / r[M:M + 1, 0:BL]).T  # [4, 64]
        out[core * BL:(core + 1) * BL, 0, 0:P] = hv
        out[core * BL:(core + 1) * BL, 0, P:P + M] = ctxv
    return out
